# revision 25
# baseline (speedup 1.0000x reference)
"""Trainium2 Bass kernel for cosine linear-attention (nn_Attention).

Data-parallel over batch N=16 across 8 NeuronCores (2 batches/core,
weights replicated, no collectives). Per core:

  q = l2norm(x @ Wq.T), k = l2norm(x @ Wk.T), v = (x @ Wv.T) * C^-sigmoid(nc)
  out = (q @ (k^T v per head)) @ Wo.T

Compute runs in bf16 on the TensorEngine (1 cyc/row vs 4 for f32), f32
PSUM accumulation. x and the weights are cast f32->bf16 via SWDGE DMA
into DRAM scratch, then DMA-transposed (X-bar) into SBUF so the
contraction dim (d) lands on partitions. The k-l2norm scale and the
per-head v scale are folded into v; q's l2norm scale is applied to q
before its PE transpose into q^T layout for the attention matmuls.
"""

import sys

for _p in ("/opt/trn_rl_repo",):
    if _p not in sys.path:
        sys.path.append(_p)

import numpy as np
from contextlib import ExitStack

import concourse.bass as bass
import concourse.tile as tile
from concourse import bacc, mybir
from concourse.masks import make_identity
from concourse.bass_utils import run_bass_kernel_spmd

F32 = mybir.dt.float32
BF16 = mybir.dt.bfloat16

N_CORES = 8
N, C, D = 16, 1024, 1024
H, HD = 16, 64
B = N // N_CORES          # batches per core
P = 128
KC = D // P               # contraction chunks (8)
CT = C // P               # c tiles per batch (8)
MC = D // 512             # 512-wide m chunks (2)
HP = H // 2               # head pairs (8)
LN_C = float(np.log(C))


def build_graph():
    nc = bacc.Bacc("TRN2", target_bir_lowering=False, debug=False,
                   num_devices=N_CORES)
    x_ext = nc.declare_dram_parameter("x", [B, C, D], F32, isOutput=False)
    w_ext = {
        w: nc.declare_dram_parameter(w, [D, D], F32, isOutput=False)
        for w in ("Wq", "Wk", "Wv", "Wo")
    }
    ncst_ext = nc.declare_dram_parameter("norm_const", [1, H, 1, 1], F32,
                                         isOutput=False)
    out_ext = nc.declare_dram_parameter("out", [B, C, D], F32, isOutput=True)

    with tile.TileContext(nc) as tc, ExitStack() as ctx:
        singles = ctx.enter_context(tc.tile_pool(name="singles", bufs=1))
        dram = ctx.enter_context(tc.tile_pool(name="dram", bufs=1,
                                              space="DRAM"))
        ld_pool = ctx.enter_context(tc.tile_pool(name="ld", bufs=2))
        cast_pool = ctx.enter_context(tc.tile_pool(name="cast", bufs=2))
        wt_pool = ctx.enter_context(tc.tile_pool(name="wt", bufs=1))
        xt_pool = ctx.enter_context(tc.tile_pool(name="xt", bufs=1))
        kvq_pool = ctx.enter_context(tc.tile_pool(name="kvq", bufs=2))
        sq_pool = ctx.enter_context(tc.tile_pool(name="sq", bufs=2))
        stat_pool = ctx.enter_context(tc.tile_pool(name="stat", bufs=2))
        qt_pool = ctx.enter_context(tc.tile_pool(name="qt", bufs=1))
        at_pool = ctx.enter_context(tc.tile_pool(name="at", bufs=1))
        bd_pool = ctx.enter_context(tc.tile_pool(name="bd", bufs=2))
        out_pool = ctx.enter_context(tc.tile_pool(name="osb", bufs=3))
        proj_psum = ctx.enter_context(
            tc.tile_pool(name="proj_psum", bufs=6, space="PSUM"))
        kv_sb_pool = ctx.enter_context(tc.tile_pool(name="kvacc", bufs=2))
        tp_psum = ctx.enter_context(
            tc.tile_pool(name="tp_psum", bufs=2, space="PSUM"))

        # ---- prologue: per-head v scale C^-sigmoid(norm_const) -> [128, H]
        svec = singles.tile([1, H], F32, name="svec")
        nc.sync.dma_start(out=svec[:], in_=ncst_ext[0, :, 0, 0])
        ssig = singles.tile([1, H], F32, name="ssig")
        nc.scalar.activation(ssig[:], svec[:],
                             mybir.ActivationFunctionType.Sigmoid)
        sexp = singles.tile([1, H], F32, name="sexp")
        nc.scalar.activation(sexp[:], ssig[:],
                             mybir.ActivationFunctionType.Exp, scale=-LN_C)
        sv128 = singles.tile([P, H], F32, name="sv128")
        nc.gpsimd.partition_broadcast(sv128[:], sexp[0:1, :])

        ident = singles.tile([P, P], BF16, name="ident")
        make_identity(nc, ident[:])

        # ---- weights into SBUF transposed: wt[w][kc] = W.T[kc*128:.., :].
        # Wk goes through the PE (load f32 rows, ACT cast, PE transpose) so
        # the K projections can start within a few us; Wv/Wq/Wo take the
        # DMA route (whole-W SWDGE cast to DRAM bf16, then X-bar
        # DMA-transpose), overlapped under the K pass compute.
        wt = {
            w: [
                wt_pool.tile([P, D], BF16, name=f"wt_{w}_{kc}",
                             tag=f"wt_{w}_{kc}")
                for kc in range(KC)
            ]
            for w in ("Wk", "Wv", "Wq", "Wo")
        }
        for wname in ("Wk", "Wv"):
            for mt in range(KC):
                wf = ld_pool.tile([P, D], F32, name="wf", tag="wf")
                nc.sync.dma_start(out=wf[:],
                                  in_=w_ext[wname][mt * P:(mt + 1) * P, :])
                wb = cast_pool.tile([P, D], BF16, name="wb", tag="wb")
                nc.scalar.copy(wb[:], wf[:])
                for kc in range(KC):
                    pst = tp_psum.tile([P, P], BF16, name="wpst", tag="pst")
                    nc.tensor.transpose(pst[:], wb[:, kc * P:(kc + 1) * P],
                                        ident[:])
                    nc.vector.tensor_copy(
                        wt[wname][kc][:, mt * P:(mt + 1) * P], pst[:])

        def dma_weight(wname):
            # whole-W SWDGE cast to DRAM bf16, then X-bar transposes; emitted
            # late so its HBM traffic stays off the startup critical path
            wbf = dram.tile([D, D], BF16, name=f"wbf_{wname}",
                            tag=f"wbf_{wname}")
            nc.gpsimd.dma_start(out=wbf[:], in_=w_ext[wname][:, :])
            for kc in range(KC):
                nc.sync.dma_start(out=wt[wname][kc][:],
                                  in_=wbf[:, kc * P:(kc + 1) * P],
                                  transpose=True)

        for n in range(B):
            # ---- x: same load/cast/PE-transpose; xt[kc] = x[n].T chunk
            xts = [
                xt_pool.tile([P, C], BF16, name=f"xt_{kc}", tag=f"xt_{kc}")
                for kc in range(KC)
            ]
            for ct_ in range(CT):
                xf = ld_pool.tile([P, D], F32, name="xf", tag="xf")
                nc.scalar.dma_start(out=xf[:],
                                    in_=x_ext[n, ct_ * P:(ct_ + 1) * P, :])
                xb = cast_pool.tile([P, D], BF16, name="xb", tag="xb")
                nc.scalar.copy(xb[:], xf[:])
                for kc in range(KC):
                    pst = tp_psum.tile([P, P], BF16, name="xpst", tag="pst")
                    nc.tensor.transpose(pst[:], xb[:, kc * P:(kc + 1) * P],
                                        ident[:])
                    nc.vector.tensor_copy(
                        xts[kc][:, ct_ * P:(ct_ + 1) * P], pst[:])

            # per-(head-pair) kv accumulators in SBUF f32 (PSUM accumulation
            # groups can't interleave within a bank: start=True clears
            # has_written for the whole 2KB zero region). Each c tile's kv
            # partial is a start+stop matmul into PSUM, then DVE-added here.
            kvsb = [
                kv_sb_pool.tile([P, 512], F32, name=f"kvsb_{b}",
                                tag=f"kvsb_{b}")
                for b in range(2)
            ]

            # q^T strips, written chunk-by-chunk across the c loop
            qts = [
                qt_pool.tile([P, C], BF16, name=f"qt_{mt}", tag=f"qt_{mt}")
                for mt in range(KC)
            ]

            # ---- phase A helpers
            def project(wname, ct, pname):
                cs = slice(ct * P, (ct + 1) * P)
                ps = {}
                for mc in range(MC):
                    ps[mc] = proj_psum.tile([P, 512], F32,
                                            name=f"ps{pname}_{mc}",
                                            tag="proj")
                for kc in range(KC):
                    for mc in range(MC):
                        nc.tensor.matmul(
                            ps[mc][:],
                            xts[kc][:, cs],
                            wt[wname][kc][:, mc * 512:(mc + 1) * 512],
                            start=(kc == 0),
                            stop=(kc == KC - 1),
                        )
                return ps

            def group_sumsq(ps, ssname):
                ss = stat_pool.tile([P, H], F32, name=ssname, tag=ssname)
                for mc in range(MC):
                    sq = sq_pool.tile([P, 512], F32, name="sq", tag="sq")
                    nc.scalar.square(sq[:], ps[mc][:])
                    nc.vector.tensor_reduce(
                        ss[:, mc * 8:(mc + 1) * 8],
                        sq[:].rearrange("p (g d) -> p g d", g=8),
                        mybir.AxisListType.X,
                        mybir.AluOpType.add,
                    )
                return ss

            def rsqrt_(ss, rname):
                r = stat_pool.tile([P, H], F32, name=rname, tag=rname)
                nc.vector.tensor_scalar_max(r[:], ss[:], 1e-30)
                nc.vector.reciprocal(r[:], r[:])
                nc.scalar.sqrt(r[:], r[:])
                return r

            def scaled_to_bf16(ps, r, outname, tag=None):
                o = kvq_pool.tile([P, D], BF16, name=outname, tag=tag or outname)
                for mc in range(MC):
                    ms = slice(mc * 512, (mc + 1) * 512)
                    nc.vector.tensor_mul(
                        o[:, ms].rearrange("p (g d) -> p g d", g=8),
                        ps[mc][:].rearrange("p (g d) -> p g d", g=8),
                        r[:, mc * 8:(mc + 1) * 8][:, :, None]
                        .broadcast_to((P, 8, HD)),
                    )
                return o

            # ---- phase A-K: K projections (raw bf16; l2norm folded into v)
            ksbs, ssks = [], []
            for ct in range(CT):
                psK = project("Wk", ct, "K")
                ssks.append(group_sumsq(psK, f"ssk_{ct}"))
                ksb = kvq_pool.tile([P, D], BF16, name=f"ksb_{ct}",
                                    tag=f"ksb_{ct}")
                for mc in range(MC):
                    ms = slice(mc * 512, (mc + 1) * 512)
                    nc.any.tensor_copy(ksb[:, ms], psK[mc][:])
                ksbs.append(ksb)

            if n == 0:
                dma_weight("Wq")

            # ---- phase A-V: V projections + kv partial accumulation
            for ct in range(CT):
                psV = project("Wv", ct, "V")
                rk = rsqrt_(ssks[ct], "rk")
                rkv = stat_pool.tile([P, H], F32, name="rkv", tag="rkv")
                nc.vector.tensor_mul(rkv[:], rk[:], sv128[:])
                vsb = scaled_to_bf16(psV, rkv, "vsb")
                for b in range(2):
                    kvp = proj_psum.tile([P, 512], F32, name=f"kvp_{b}",
                                         tag="proj")
                    for j in range(4):
                        hp = b * 4 + j
                        hs = slice(hp * P, (hp + 1) * P)
                        nc.tensor.matmul(
                            kvp[:, j * P:(j + 1) * P],
                            ksbs[ct][:, hs],
                            vsb[:, hs],
                            start=True,
                            stop=True,
                        )
                    if ct == 0:
                        nc.vector.tensor_copy(kvsb[b][:], kvp[:])
                    else:
                        nc.vector.tensor_add(kvsb[b][:], kvsb[b][:], kvp[:])

            if n == 0:
                dma_weight("Wo")

            # ---- phase A-Q: Q projections + l2norm + PE transpose into q^T
            for ct in range(CT):
                cs = slice(ct * P, (ct + 1) * P)
                psQ = project("Wq", ct, "Q")
                ssq = group_sumsq(psQ, "ssq")
                rq = rsqrt_(ssq, "rq")
                qsb = scaled_to_bf16(psQ, rq, "qsb")
                for mt in range(KC):
                    pst = tp_psum.tile([P, P], BF16, name="pst", tag="pst")
                    nc.tensor.transpose(pst[:], qsb[:, mt * P:(mt + 1) * P],
                                        ident[:])
                    nc.any.tensor_copy(qts[mt][:, cs], pst[:])

            # ---- phase B: block-diagonal kv tiles (off-diag junk zeroed)
            bds = []
            for hp in range(HP):
                kv = kvsb[hp // 4][:, (hp % 4) * P:(hp % 4 + 1) * P]
                bd = bd_pool.tile([P, P], BF16, name=f"bd_{hp}", tag="bd")
                nc.gpsimd.memset(bd[:], 0.0)
                nc.gpsimd.tensor_copy(bd[0:64, 0:64], kv[0:64, 0:64])
                nc.gpsimd.tensor_copy(bd[64:P, 64:P], kv[64:P, 64:P])
                bds.append(bd)

            # ---- phase C: attn^T = blockdiag(kv)^T-free @ q^T
            ats = []
            for hp in range(HP):
                at = at_pool.tile([P, C], BF16, name=f"at_{hp}",
                                  tag=f"at_{hp}")
                for cc in range(MC):
                    ccs = slice(cc * 512, (cc + 1) * 512)
                    psA = proj_psum.tile([P, 512], F32, name="psA", tag="proj")
                    nc.tensor.matmul(psA[:], bds[hp][:], qts[hp][:, ccs],
                                     start=True, stop=True)
                    nc.any.tensor_copy(at[:, ccs], psA[:])
                ats.append(at)

            # ---- phase D: out = attn^T.T @ Wo.T (one stationary load per
            # (ct, hp) feeds both 512-wide m chunks)
            for ct in range(CT):
                cs = slice(ct * P, (ct + 1) * P)
                psO = [
                    proj_psum.tile([P, 512], F32, name=f"psO_{mc}",
                                   tag="proj")
                    for mc in range(MC)
                ]
                for hp in range(HP):
                    for mc in range(MC):
                        nc.tensor.matmul(
                            psO[mc][:],
                            ats[hp][:, cs],
                            wt["Wo"][hp][:, mc * 512:(mc + 1) * 512],
                            start=(hp == 0),
                            stop=(hp == HP - 1),
                        )
                for mc in range(MC):
                    ms = slice(mc * 512, (mc + 1) * 512)
                    osb = out_pool.tile([P, 512], F32, name="osb", tag="osb")
                    nc.any.tensor_copy(osb[:], psO[mc][:])
                    nc.sync.dma_start(out=out_ext[n, cs, ms], in_=osb[:])

    nc.compile()
    return nc


_NC_CACHE = None


def _get_graph():
    global _NC_CACHE
    if _NC_CACHE is None:
        _NC_CACHE = build_graph()
    return _NC_CACHE


def kernel(x, Wq, Wk, Wv, Wo, norm_const, _trace=False):
    x = np.ascontiguousarray(np.asarray(x, dtype=np.float32))
    Wq = np.ascontiguousarray(np.asarray(Wq, dtype=np.float32))
    Wk = np.ascontiguousarray(np.asarray(Wk, dtype=np.float32))
    Wv = np.ascontiguousarray(np.asarray(Wv, dtype=np.float32))
    Wo = np.ascontiguousarray(np.asarray(Wo, dtype=np.float32))
    norm_const = np.ascontiguousarray(np.asarray(norm_const, dtype=np.float32))

    nc = _get_graph()
    in_maps = []
    for c in range(N_CORES):
        in_maps.append({
            "x": x[c * B:(c + 1) * B],
            "Wq": Wq, "Wk": Wk, "Wv": Wv, "Wo": Wo,
            "norm_const": norm_const,
        })
    res = run_bass_kernel_spmd(nc, in_maps, list(range(N_CORES)),
                               trace=_trace)
    out = np.concatenate([res.results[c]["out"] for c in range(N_CORES)],
                         axis=0)
    if _trace:
        kernel.last_exec_time_ns = res.exec_time_ns
        kernel.last_results = res
    return out


# revision 26
# speedup vs baseline: 1.0578x; 1.0578x over previous
"""Trainium2 Bass kernel for cosine linear-attention (nn_Attention).

Data-parallel over batch N=16 across 8 NeuronCores (2 batches/core,
weights replicated, no collectives). Per core:

  q = l2norm(x @ Wq.T), k = l2norm(x @ Wk.T), v = (x @ Wv.T) * C^-sigmoid(nc)
  out = (q @ (k^T v per head)) @ Wo.T

Compute runs in bf16 on the TensorEngine (1 cyc/row vs 4 for f32), f32
PSUM accumulation. x and the weights are cast f32->bf16 via SWDGE DMA
into DRAM scratch, then DMA-transposed (X-bar) into SBUF so the
contraction dim (d) lands on partitions. The k-l2norm scale and the
per-head v scale are folded into v; q's l2norm scale is applied to q
before its PE transpose into q^T layout for the attention matmuls.
"""

import sys

for _p in ("/opt/trn_rl_repo",):
    if _p not in sys.path:
        sys.path.append(_p)

import numpy as np
from contextlib import ExitStack

import concourse.bass as bass
import concourse.tile as tile
from concourse import bacc, mybir
from concourse.masks import make_identity
from concourse.bass_utils import run_bass_kernel_spmd

F32 = mybir.dt.float32
BF16 = mybir.dt.bfloat16

N_CORES = 8
N, C, D = 16, 1024, 1024
H, HD = 16, 64
B = N // N_CORES          # batches per core
P = 128
KC = D // P               # contraction chunks (8)
CT = C // P               # c tiles per batch (8)
MC = D // 512             # 512-wide m chunks (2)
HP = H // 2               # head pairs (8)
LN_C = float(np.log(C))


def build_graph():
    nc = bacc.Bacc("TRN2", target_bir_lowering=False, debug=False,
                   num_devices=N_CORES)
    x_ext = nc.declare_dram_parameter("x", [B, C, D], F32, isOutput=False)
    w_ext = {
        w: nc.declare_dram_parameter(w, [D, D], F32, isOutput=False)
        for w in ("Wq", "Wk", "Wv", "Wo")
    }
    ncst_ext = nc.declare_dram_parameter("norm_const", [1, H, 1, 1], F32,
                                         isOutput=False)
    out_ext = nc.declare_dram_parameter("out", [B, C, D], F32, isOutput=True)

    with tile.TileContext(nc) as tc, ExitStack() as ctx:
        singles = ctx.enter_context(tc.tile_pool(name="singles", bufs=1))
        dram = ctx.enter_context(tc.tile_pool(name="dram", bufs=1,
                                              space="DRAM"))
        ld_pool = ctx.enter_context(tc.tile_pool(name="ld", bufs=2))
        cast_pool = ctx.enter_context(tc.tile_pool(name="cast", bufs=2))
        wt_pool = ctx.enter_context(tc.tile_pool(name="wt", bufs=1))
        xt_pool = ctx.enter_context(tc.tile_pool(name="xt", bufs=1))
        kvq_pool = ctx.enter_context(tc.tile_pool(name="kvq", bufs=2))
        sq_pool = ctx.enter_context(tc.tile_pool(name="sq", bufs=2))
        stat_pool = ctx.enter_context(tc.tile_pool(name="stat", bufs=2))
        qt_pool = ctx.enter_context(tc.tile_pool(name="qt", bufs=1))
        at_pool = ctx.enter_context(tc.tile_pool(name="at", bufs=1))
        bd_pool = ctx.enter_context(tc.tile_pool(name="bd", bufs=2))
        out_pool = ctx.enter_context(tc.tile_pool(name="osb", bufs=3))
        proj_psum = ctx.enter_context(
            tc.tile_pool(name="proj_psum", bufs=6, space="PSUM"))
        kv_sb_pool = ctx.enter_context(tc.tile_pool(name="kvacc", bufs=2))
        tp_psum = ctx.enter_context(
            tc.tile_pool(name="tp_psum", bufs=2, space="PSUM"))

        # ---- prologue: per-head v scale C^-sigmoid(norm_const) -> [128, H]
        svec = singles.tile([1, H], F32, name="svec")
        nc.sync.dma_start(out=svec[:], in_=ncst_ext[0, :, 0, 0])
        ssig = singles.tile([1, H], F32, name="ssig")
        nc.scalar.activation(ssig[:], svec[:],
                             mybir.ActivationFunctionType.Sigmoid)
        sexp = singles.tile([1, H], F32, name="sexp")
        nc.scalar.activation(sexp[:], ssig[:],
                             mybir.ActivationFunctionType.Exp, scale=-LN_C)
        sv128 = singles.tile([P, H], F32, name="sv128")
        nc.gpsimd.partition_broadcast(sv128[:], sexp[0:1, :])

        ident = singles.tile([P, P], BF16, name="ident")
        make_identity(nc, ident[:])

        # ---- weights into SBUF transposed: wt[w][kc] = W.T[kc*128:.., :].
        # Wk goes through the PE (load f32 rows, ACT cast, PE transpose) so
        # the K projections can start within a few us; Wv/Wq/Wo take the
        # DMA route (whole-W SWDGE cast to DRAM bf16, then X-bar
        # DMA-transpose), overlapped under the K pass compute.
        wt = {
            w: wt_pool.tile([P, KC, D], BF16, name=f"wt_{w}", tag=f"wt_{w}")
            for w in ("Wk", "Wv", "Wq", "Wo")
        }
        for wname in ("Wk", "Wv"):
            for mt in range(KC):
                wf = ld_pool.tile([P, D], F32, name="wf", tag="wf", bufs=3)
                nc.sync.dma_start(out=wf[:],
                                  in_=w_ext[wname][mt * P:(mt + 1) * P, :])
                wb = cast_pool.tile([P, D], BF16, name="wb", tag="wb", bufs=3)
                nc.scalar.copy(wb[:], wf[:])
                for g in range(2):
                    pst = tp_psum.tile([P, 512], BF16, name="wpst", tag="pst")
                    for j in range(4):
                        kc = g * 4 + j
                        nc.tensor.transpose(pst[:, j * P:(j + 1) * P],
                                            wb[:, kc * P:(kc + 1) * P],
                                            ident[:])
                    nc.vector.tensor_copy(
                        wt[wname][:, g * 4:(g + 1) * 4,
                                  mt * P:(mt + 1) * P],
                        pst[:].rearrange("p (j m) -> p j m", j=4))

        def dma_weight(wname):
            # whole-W SWDGE cast to DRAM bf16, then X-bar transposes; emitted
            # late so its HBM traffic stays off the startup critical path
            wbf = dram.tile([D, D], BF16, name=f"wbf_{wname}",
                            tag=f"wbf_{wname}")
            nc.gpsimd.dma_start(out=wbf[:], in_=w_ext[wname][:, :])
            for kc in range(KC):
                nc.sync.dma_start(out=wt[wname][:, kc, :],
                                  in_=wbf[:, kc * P:(kc + 1) * P],
                                  transpose=True)

        for n in range(B):
            # ---- x: same load/cast/PE-transpose; xt[kc] = x[n].T chunk
            xt = xt_pool.tile([P, KC, C], BF16, name="xt", tag="xt")
            for ct_ in range(CT):
                xf = ld_pool.tile([P, D], F32, name="xf", tag="xf", bufs=3)
                nc.scalar.dma_start(out=xf[:],
                                    in_=x_ext[n, ct_ * P:(ct_ + 1) * P, :])
                xb = cast_pool.tile([P, D], BF16, name="xb", tag="xb", bufs=3)
                nc.scalar.copy(xb[:], xf[:])
                for g in range(2):
                    pst = tp_psum.tile([P, 512], BF16, name="xpst", tag="pst")
                    for j in range(4):
                        kc = g * 4 + j
                        nc.tensor.transpose(pst[:, j * P:(j + 1) * P],
                                            xb[:, kc * P:(kc + 1) * P],
                                            ident[:])
                    nc.vector.tensor_copy(
                        xt[:, g * 4:(g + 1) * 4, ct_ * P:(ct_ + 1) * P],
                        pst[:].rearrange("p (j m) -> p j m", j=4))

            # per-(head-pair) kv accumulators in SBUF f32 (PSUM accumulation
            # groups can't interleave within a bank: start=True clears
            # has_written for the whole 2KB zero region). Each c tile's kv
            # partial is a start+stop matmul into PSUM, then DVE-added here.
            kvsb = [
                kv_sb_pool.tile([P, 512], F32, name=f"kvsb_{b}",
                                tag=f"kvsb_{b}")
                for b in range(2)
            ]

            # q^T strips, written chunk-by-chunk across the c loop
            qt = qt_pool.tile([P, KC, C], BF16, name="qt", tag="qt")

            # ---- phase A helpers
            def project(wname, ct, pname):
                cs = slice(ct * P, (ct + 1) * P)
                ps = {}
                for mc in range(MC):
                    ps[mc] = proj_psum.tile([P, 512], F32,
                                            name=f"ps{pname}_{mc}",
                                            tag="proj")
                for kc in range(KC):
                    for mc in range(MC):
                        nc.tensor.matmul(
                            ps[mc][:],
                            xt[:, kc, cs],
                            wt[wname][:, kc, mc * 512:(mc + 1) * 512],
                            start=(kc == 0),
                            stop=(kc == KC - 1),
                        )
                return ps

            def group_sumsq(ps, ssname):
                ss = stat_pool.tile([P, H], F32, name=ssname, tag=ssname)
                for mc in range(MC):
                    sq = sq_pool.tile([P, 512], F32, name="sq", tag="sq")
                    nc.scalar.square(sq[:], ps[mc][:])
                    nc.vector.tensor_reduce(
                        ss[:, mc * 8:(mc + 1) * 8],
                        sq[:].rearrange("p (g d) -> p g d", g=8),
                        mybir.AxisListType.X,
                        mybir.AluOpType.add,
                    )
                return ss

            def rsqrt_(ss, rname):
                r = stat_pool.tile([P, H], F32, name=rname, tag=rname)
                nc.vector.tensor_scalar_max(r[:], ss[:], 1e-30)
                nc.vector.reciprocal(r[:], r[:])
                nc.scalar.sqrt(r[:], r[:])
                return r

            def scaled_to_bf16(ps, r, outname, tag=None):
                o = kvq_pool.tile([P, D], BF16, name=outname, tag=tag or outname)
                for mc in range(MC):
                    ms = slice(mc * 512, (mc + 1) * 512)
                    nc.vector.tensor_mul(
                        o[:, ms].rearrange("p (g d) -> p g d", g=8),
                        ps[mc][:].rearrange("p (g d) -> p g d", g=8),
                        r[:, mc * 8:(mc + 1) * 8][:, :, None]
                        .broadcast_to((P, 8, HD)),
                    )
                return o

            # ---- phase A-K: K projections (raw bf16; l2norm folded into v)
            ksbs, ssks = [], []
            for ct in range(CT):
                psK = project("Wk", ct, "K")
                ssks.append(group_sumsq(psK, f"ssk_{ct}"))
                ksb = kvq_pool.tile([P, D], BF16, name=f"ksb_{ct}",
                                    tag=f"ksb_{ct}", bufs=1)
                for mc in range(MC):
                    ms = slice(mc * 512, (mc + 1) * 512)
                    nc.any.tensor_copy(ksb[:, ms], psK[mc][:])
                ksbs.append(ksb)

            if n == 0:
                dma_weight("Wq")

            # ---- phase A-V: V projections + kv partial accumulation
            for ct in range(CT):
                psV = project("Wv", ct, "V")
                rk = rsqrt_(ssks[ct], "rk")
                rkv = stat_pool.tile([P, H], F32, name="rkv", tag="rkv")
                nc.vector.tensor_mul(rkv[:], rk[:], sv128[:])
                vsb = scaled_to_bf16(psV, rkv, "vsb")
                for b in range(2):
                    kvp = proj_psum.tile([P, 512], F32, name=f"kvp_{b}",
                                         tag="proj")
                    for j in range(4):
                        hp = b * 4 + j
                        hs = slice(hp * P, (hp + 1) * P)
                        nc.tensor.matmul(
                            kvp[:, j * P:(j + 1) * P],
                            ksbs[ct][:, hs],
                            vsb[:, hs],
                            start=True,
                            stop=True,
                        )
                    if ct == 0:
                        nc.vector.tensor_copy(kvsb[b][:], kvp[:])
                    else:
                        nc.vector.tensor_add(kvsb[b][:], kvsb[b][:], kvp[:])

            if n == 0:
                dma_weight("Wo")

            # ---- phase A-Q: Q projections + l2norm + PE transpose into q^T
            for ct in range(CT):
                cs = slice(ct * P, (ct + 1) * P)
                psQ = project("Wq", ct, "Q")
                ssq = group_sumsq(psQ, "ssq")
                rq = rsqrt_(ssq, "rq")
                qsb = scaled_to_bf16(psQ, rq, "qsb")
                for g in range(2):
                    pst = tp_psum.tile([P, 512], BF16, name="pst", tag="pst")
                    for j in range(4):
                        mt = g * 4 + j
                        nc.tensor.transpose(pst[:, j * P:(j + 1) * P],
                                            qsb[:, mt * P:(mt + 1) * P],
                                            ident[:])
                    nc.any.tensor_copy(
                        qt[:, g * 4:(g + 1) * 4, cs],
                        pst[:].rearrange("p (j m) -> p j m", j=4))

            # ---- phase B: block-diagonal kv tiles (off-diag junk zeroed)
            bds = []
            for hp in range(HP):
                kv = kvsb[hp // 4][:, (hp % 4) * P:(hp % 4 + 1) * P]
                bd = bd_pool.tile([P, P], BF16, name=f"bd_{hp}", tag="bd")
                nc.gpsimd.memset(bd[:], 0.0)
                nc.gpsimd.tensor_copy(bd[0:64, 0:64], kv[0:64, 0:64])
                nc.gpsimd.tensor_copy(bd[64:P, 64:P], kv[64:P, 64:P])
                bds.append(bd)

            # ---- phase C: attn^T = blockdiag(kv)^T-free @ q^T
            ats = []
            for hp in range(HP):
                at = at_pool.tile([P, C], BF16, name=f"at_{hp}",
                                  tag=f"at_{hp}")
                for cc in range(MC):
                    ccs = slice(cc * 512, (cc + 1) * 512)
                    psA = proj_psum.tile([P, 512], F32, name="psA", tag="proj")
                    nc.tensor.matmul(psA[:], bds[hp][:], qt[:, hp, ccs],
                                     start=True, stop=True)
                    nc.any.tensor_copy(at[:, ccs], psA[:])
                ats.append(at)

            # ---- phase D: out = attn^T.T @ Wo.T (one stationary load per
            # (ct, hp) feeds both 512-wide m chunks)
            for ct in range(CT):
                cs = slice(ct * P, (ct + 1) * P)
                psO = [
                    proj_psum.tile([P, 512], F32, name=f"psO_{mc}",
                                   tag="proj")
                    for mc in range(MC)
                ]
                for hp in range(HP):
                    for mc in range(MC):
                        nc.tensor.matmul(
                            psO[mc][:],
                            ats[hp][:, cs],
                            wt["Wo"][:, hp, mc * 512:(mc + 1) * 512],
                            start=(hp == 0),
                            stop=(hp == HP - 1),
                        )
                for mc in range(MC):
                    ms = slice(mc * 512, (mc + 1) * 512)
                    osb = out_pool.tile([P, 512], F32, name="osb", tag="osb")
                    nc.any.tensor_copy(osb[:], psO[mc][:])
                    nc.sync.dma_start(out=out_ext[n, cs, ms], in_=osb[:])

    nc.compile()
    return nc


_NC_CACHE = None


def _get_graph():
    global _NC_CACHE
    if _NC_CACHE is None:
        _NC_CACHE = build_graph()
    return _NC_CACHE


def kernel(x, Wq, Wk, Wv, Wo, norm_const, _trace=False):
    x = np.ascontiguousarray(np.asarray(x, dtype=np.float32))
    Wq = np.ascontiguousarray(np.asarray(Wq, dtype=np.float32))
    Wk = np.ascontiguousarray(np.asarray(Wk, dtype=np.float32))
    Wv = np.ascontiguousarray(np.asarray(Wv, dtype=np.float32))
    Wo = np.ascontiguousarray(np.asarray(Wo, dtype=np.float32))
    norm_const = np.ascontiguousarray(np.asarray(norm_const, dtype=np.float32))

    nc = _get_graph()
    in_maps = []
    for c in range(N_CORES):
        in_maps.append({
            "x": x[c * B:(c + 1) * B],
            "Wq": Wq, "Wk": Wk, "Wv": Wv, "Wo": Wo,
            "norm_const": norm_const,
        })
    res = run_bass_kernel_spmd(nc, in_maps, list(range(N_CORES)),
                               trace=_trace)
    out = np.concatenate([res.results[c]["out"] for c in range(N_CORES)],
                         axis=0)
    if _trace:
        kernel.last_exec_time_ns = res.exec_time_ns
        kernel.last_results = res
    return out


# revision 27
# speedup vs baseline: 1.1122x; 1.0515x over previous
"""Trainium2 Bass kernel for cosine linear-attention (nn_Attention).

Data-parallel over batch N=16 across 8 NeuronCores (2 batches/core,
weights replicated, no collectives). Per core:

  q = l2norm(x @ Wq.T), k = l2norm(x @ Wk.T), v = (x @ Wv.T) * C^-sigmoid(nc)
  out = (q @ (k^T v per head)) @ Wo.T

Compute runs in bf16 on the TensorEngine (1 cyc/row vs 4 for f32), f32
PSUM accumulation. x and the weights are cast f32->bf16 via SWDGE DMA
into DRAM scratch, then DMA-transposed (X-bar) into SBUF so the
contraction dim (d) lands on partitions. The k-l2norm scale and the
per-head v scale are folded into v; q's l2norm scale is applied to q
before its PE transpose into q^T layout for the attention matmuls.
"""

import sys

for _p in ("/opt/trn_rl_repo",):
    if _p not in sys.path:
        sys.path.append(_p)

import numpy as np
from contextlib import ExitStack

import concourse.bass as bass
import concourse.tile as tile
from concourse import bacc, mybir
from concourse.masks import make_identity
from concourse.bass_utils import run_bass_kernel_spmd

F32 = mybir.dt.float32
BF16 = mybir.dt.bfloat16

N_CORES = 8
N, C, D = 16, 1024, 1024
H, HD = 16, 64
B = N // N_CORES          # batches per core
P = 128
KC = D // P               # contraction chunks (8)
CT = C // P               # c tiles per batch (8)
MC = D // 512             # 512-wide m chunks (2)
HP = H // 2               # head pairs (8)
LN_C = float(np.log(C))


def build_graph():
    nc = bacc.Bacc("TRN2", target_bir_lowering=False, debug=False,
                   num_devices=N_CORES)
    x_ext = nc.declare_dram_parameter("x", [B, C, D], F32, isOutput=False)
    w_ext = {
        w: nc.declare_dram_parameter(w, [D, D], F32, isOutput=False)
        for w in ("Wq", "Wk", "Wv", "Wo")
    }
    ncst_ext = nc.declare_dram_parameter("norm_const", [1, H, 1, 1], F32,
                                         isOutput=False)
    out_ext = nc.declare_dram_parameter("out", [B, C, D], F32, isOutput=True)

    with tile.TileContext(nc) as tc, ExitStack() as ctx:
        singles = ctx.enter_context(tc.tile_pool(name="singles", bufs=1))
        dram = ctx.enter_context(tc.tile_pool(name="dram", bufs=1,
                                              space="DRAM"))
        ld_pool = ctx.enter_context(tc.tile_pool(name="ld", bufs=2))
        cast_pool = ctx.enter_context(tc.tile_pool(name="cast", bufs=2))
        wt_pool = ctx.enter_context(tc.tile_pool(name="wt", bufs=1))
        xt_pool = ctx.enter_context(tc.tile_pool(name="xt", bufs=1))
        kvq_pool = ctx.enter_context(tc.tile_pool(name="kvq", bufs=2))
        sq_pool = ctx.enter_context(tc.tile_pool(name="sq", bufs=2))
        stat_pool = ctx.enter_context(tc.tile_pool(name="stat", bufs=2))
        qt_pool = ctx.enter_context(tc.tile_pool(name="qt", bufs=1))
        at_pool = ctx.enter_context(tc.tile_pool(name="at", bufs=1))
        bd_pool = ctx.enter_context(tc.tile_pool(name="bd", bufs=2))
        out_pool = ctx.enter_context(tc.tile_pool(name="osb", bufs=3))
        proj_psum = ctx.enter_context(
            tc.tile_pool(name="proj_psum", bufs=6, space="PSUM"))
        kv_sb_pool = ctx.enter_context(tc.tile_pool(name="kvacc", bufs=2))
        tp_psum = ctx.enter_context(
            tc.tile_pool(name="tp_psum", bufs=2, space="PSUM"))

        # ---- prologue: per-head v scale C^-sigmoid(norm_const) -> [128, H]
        svec = singles.tile([1, H], F32, name="svec")
        nc.sync.dma_start(out=svec[:], in_=ncst_ext[0, :, 0, 0])
        ssig = singles.tile([1, H], F32, name="ssig")
        nc.scalar.activation(ssig[:], svec[:],
                             mybir.ActivationFunctionType.Sigmoid)
        sexp = singles.tile([1, H], F32, name="sexp")
        nc.scalar.activation(sexp[:], ssig[:],
                             mybir.ActivationFunctionType.Exp, scale=-LN_C)
        sv128 = singles.tile([P, H], F32, name="sv128")
        nc.gpsimd.partition_broadcast(sv128[:], sexp[0:1, :])

        ident = singles.tile([P, P], BF16, name="ident")
        make_identity(nc, ident[:])

        # ---- weights into SBUF transposed: wt[w][kc] = W.T[kc*128:.., :].
        # Wk goes through the PE (load f32 rows, ACT cast, PE transpose) so
        # the K projections can start within a few us; Wv/Wq/Wo take the
        # DMA route (whole-W SWDGE cast to DRAM bf16, then X-bar
        # DMA-transpose), overlapped under the K pass compute.
        wt = {
            w: wt_pool.tile([P, KC, D], BF16, name=f"wt_{w}", tag=f"wt_{w}")
            for w in ("Wk", "Wv", "Wq", "Wo")
        }
        for wname in ("Wk", "Wv"):
            for mt in range(KC):
                wf = ld_pool.tile([P, D], F32, name="wf", tag="wf", bufs=3)
                nc.sync.dma_start(out=wf[:],
                                  in_=w_ext[wname][mt * P:(mt + 1) * P, :])
                wb = cast_pool.tile([P, D], BF16, name="wb", tag="wb", bufs=3)
                nc.scalar.copy(wb[:], wf[:])
                for g in range(2):
                    pst = tp_psum.tile([P, 512], BF16, name="wpst", tag="pst")
                    for j in range(4):
                        kc = g * 4 + j
                        nc.tensor.transpose(pst[:, j * P:(j + 1) * P],
                                            wb[:, kc * P:(kc + 1) * P],
                                            ident[:])
                    nc.vector.tensor_copy(
                        wt[wname][:, g * 4:(g + 1) * 4,
                                  mt * P:(mt + 1) * P],
                        pst[:].rearrange("p (j m) -> p j m", j=4))

        def dma_weight(wname):
            # whole-W SWDGE cast to DRAM bf16, then X-bar transposes; emitted
            # late so its HBM traffic stays off the startup critical path
            wbf = dram.tile([D, D], BF16, name=f"wbf_{wname}",
                            tag=f"wbf_{wname}")
            nc.gpsimd.dma_start(out=wbf[:], in_=w_ext[wname][:, :])
            for kc in range(KC):
                nc.sync.dma_start(out=wt[wname][:, kc, :],
                                  in_=wbf[:, kc * P:(kc + 1) * P],
                                  transpose=True)

        for n in range(B):
            # ---- x: same load/cast/PE-transpose; xt[kc] = x[n].T chunk
            xt = xt_pool.tile([P, KC, C], BF16, name="xt", tag="xt")
            for ct_ in range(CT):
                xf = ld_pool.tile([P, D], F32, name="xf", tag="xf", bufs=4)
                nc.scalar.dma_start(out=xf[:],
                                    in_=x_ext[n, ct_ * P:(ct_ + 1) * P, :])
                xb = cast_pool.tile([P, D], BF16, name="xb", tag="xb", bufs=3)
                nc.scalar.copy(xb[:], xf[:])
                for g in range(2):
                    pst = tp_psum.tile([P, 512], BF16, name="xpst", tag="pst")
                    for j in range(4):
                        kc = g * 4 + j
                        nc.tensor.transpose(pst[:, j * P:(j + 1) * P],
                                            xb[:, kc * P:(kc + 1) * P],
                                            ident[:])
                    nc.vector.tensor_copy(
                        xt[:, g * 4:(g + 1) * 4, ct_ * P:(ct_ + 1) * P],
                        pst[:].rearrange("p (j m) -> p j m", j=4))

            # per-(head-pair) kv accumulators in SBUF f32 (PSUM accumulation
            # groups can't interleave within a bank: start=True clears
            # has_written for the whole 2KB zero region). Each c tile's kv
            # partial is a start+stop matmul into PSUM, then DVE-added here.
            kvsb = [
                kv_sb_pool.tile([P, 512], F32, name=f"kvsb_{b}",
                                tag=f"kvsb_{b}")
                for b in range(2)
            ]

            # q^T strips, written chunk-by-chunk across the c loop
            qt = qt_pool.tile([P, KC, C], BF16, name="qt", tag="qt")

            # ---- phase A helpers
            def project(wname, ct, pname):
                cs = slice(ct * P, (ct + 1) * P)
                ps = {}
                for mc in range(MC):
                    ps[mc] = proj_psum.tile([P, 512], F32,
                                            name=f"ps{pname}_{mc}",
                                            tag="proj")
                for kc in range(KC):
                    for mc in range(MC):
                        nc.tensor.matmul(
                            ps[mc][:],
                            xt[:, kc, cs],
                            wt[wname][:, kc, mc * 512:(mc + 1) * 512],
                            start=(kc == 0),
                            stop=(kc == KC - 1),
                        )
                return ps

            def group_sumsq(ps, ssname):
                ss = stat_pool.tile([P, H], F32, name=ssname, tag=ssname)
                for mc in range(MC):
                    sq = sq_pool.tile([P, 512], F32, name="sq", tag="sq")
                    nc.scalar.square(sq[:], ps[mc][:])
                    nc.vector.tensor_reduce(
                        ss[:, mc * 8:(mc + 1) * 8],
                        sq[:].rearrange("p (g d) -> p g d", g=8),
                        mybir.AxisListType.X,
                        mybir.AluOpType.add,
                    )
                return ss

            def rsqrt_(ss, rname):
                r = stat_pool.tile([P, H], F32, name=rname, tag=rname)
                nc.vector.tensor_scalar_max(r[:], ss[:], 1e-30)
                nc.vector.reciprocal(r[:], r[:])
                nc.scalar.sqrt(r[:], r[:])
                return r

            def scaled_to_bf16(ps, r, outname, tag=None):
                o = kvq_pool.tile([P, D], BF16, name=outname, tag=tag or outname)
                for mc in range(MC):
                    ms = slice(mc * 512, (mc + 1) * 512)
                    nc.vector.tensor_mul(
                        o[:, ms].rearrange("p (g d) -> p g d", g=8),
                        ps[mc][:].rearrange("p (g d) -> p g d", g=8),
                        r[:, mc * 8:(mc + 1) * 8][:, :, None]
                        .broadcast_to((P, 8, HD)),
                    )
                return o

            # ---- phase A-K: K projections (raw bf16; l2norm folded into v)
            ksbs, ssks = [], []
            for ct in range(CT):
                psK = project("Wk", ct, "K")
                ssks.append(group_sumsq(psK, f"ssk_{ct}"))
                ksb = kvq_pool.tile([P, D], BF16, name=f"ksb_{ct}",
                                    tag=f"ksb_{ct}", bufs=1)
                for mc in range(MC):
                    ms = slice(mc * 512, (mc + 1) * 512)
                    nc.any.tensor_copy(ksb[:, ms], psK[mc][:])
                ksbs.append(ksb)

            if n == 0:
                with tc.tile_wait_until(0.020):
                    dma_weight("Wq")

            # ---- phase A-V: V projections + kv partial accumulation
            for ct in range(CT):
                psV = project("Wv", ct, "V")
                rk = rsqrt_(ssks[ct], "rk")
                rkv = stat_pool.tile([P, H], F32, name="rkv", tag="rkv")
                nc.vector.tensor_mul(rkv[:], rk[:], sv128[:])
                vsb = scaled_to_bf16(psV, rkv, "vsb")
                for b in range(2):
                    kvp = proj_psum.tile([P, 512], F32, name=f"kvp_{b}",
                                         tag="proj")
                    for j in range(4):
                        hp = b * 4 + j
                        hs = slice(hp * P, (hp + 1) * P)
                        nc.tensor.matmul(
                            kvp[:, j * P:(j + 1) * P],
                            ksbs[ct][:, hs],
                            vsb[:, hs],
                            start=True,
                            stop=True,
                        )
                    if ct == 0:
                        nc.vector.tensor_copy(kvsb[b][:], kvp[:])
                    else:
                        nc.vector.tensor_add(kvsb[b][:], kvsb[b][:], kvp[:])

            if n == 0:
                with tc.tile_wait_until(0.045):
                    dma_weight("Wo")

            # ---- phase A-Q: Q projections + l2norm + PE transpose into q^T
            for ct in range(CT):
                cs = slice(ct * P, (ct + 1) * P)
                psQ = project("Wq", ct, "Q")
                ssq = group_sumsq(psQ, "ssq")
                rq = rsqrt_(ssq, "rq")
                qsb = scaled_to_bf16(psQ, rq, "qsb")
                for g in range(2):
                    pst = tp_psum.tile([P, 512], BF16, name="pst", tag="pst")
                    for j in range(4):
                        mt = g * 4 + j
                        nc.tensor.transpose(pst[:, j * P:(j + 1) * P],
                                            qsb[:, mt * P:(mt + 1) * P],
                                            ident[:])
                    nc.any.tensor_copy(
                        qt[:, g * 4:(g + 1) * 4, cs],
                        pst[:].rearrange("p (j m) -> p j m", j=4))

            # ---- phase B: block-diagonal kv tiles (off-diag junk zeroed)
            bds = []
            for hp in range(HP):
                kv = kvsb[hp // 4][:, (hp % 4) * P:(hp % 4 + 1) * P]
                bd = bd_pool.tile([P, P], BF16, name=f"bd_{hp}", tag="bd")
                nc.gpsimd.memset(bd[:], 0.0)
                nc.gpsimd.tensor_copy(bd[0:64, 0:64], kv[0:64, 0:64])
                nc.gpsimd.tensor_copy(bd[64:P, 64:P], kv[64:P, 64:P])
                bds.append(bd)

            # ---- phase C: attn^T = blockdiag(kv)^T-free @ q^T
            ats = []
            for hp in range(HP):
                at = at_pool.tile([P, C], BF16, name=f"at_{hp}",
                                  tag=f"at_{hp}")
                for cc in range(MC):
                    ccs = slice(cc * 512, (cc + 1) * 512)
                    psA = proj_psum.tile([P, 512], F32, name="psA", tag="proj")
                    nc.tensor.matmul(psA[:], bds[hp][:], qt[:, hp, ccs],
                                     start=True, stop=True)
                    nc.any.tensor_copy(at[:, ccs], psA[:])
                ats.append(at)

            # ---- phase D: out = attn^T.T @ Wo.T (one stationary load per
            # (ct, hp) feeds both 512-wide m chunks)
            for ct in range(CT):
                cs = slice(ct * P, (ct + 1) * P)
                psO = [
                    proj_psum.tile([P, 512], F32, name=f"psO_{mc}",
                                   tag="proj")
                    for mc in range(MC)
                ]
                for hp in range(HP):
                    for mc in range(MC):
                        nc.tensor.matmul(
                            psO[mc][:],
                            ats[hp][:, cs],
                            wt["Wo"][:, hp, mc * 512:(mc + 1) * 512],
                            start=(hp == 0),
                            stop=(hp == HP - 1),
                        )
                for mc in range(MC):
                    ms = slice(mc * 512, (mc + 1) * 512)
                    osb = out_pool.tile([P, 512], F32, name="osb", tag="osb")
                    nc.any.tensor_copy(osb[:], psO[mc][:])
                    nc.sync.dma_start(out=out_ext[n, cs, ms], in_=osb[:])

    nc.compile()
    return nc


_NC_CACHE = None


def _get_graph():
    global _NC_CACHE
    if _NC_CACHE is None:
        _NC_CACHE = build_graph()
    return _NC_CACHE


def kernel(x, Wq, Wk, Wv, Wo, norm_const, _trace=False):
    x = np.ascontiguousarray(np.asarray(x, dtype=np.float32))
    Wq = np.ascontiguousarray(np.asarray(Wq, dtype=np.float32))
    Wk = np.ascontiguousarray(np.asarray(Wk, dtype=np.float32))
    Wv = np.ascontiguousarray(np.asarray(Wv, dtype=np.float32))
    Wo = np.ascontiguousarray(np.asarray(Wo, dtype=np.float32))
    norm_const = np.ascontiguousarray(np.asarray(norm_const, dtype=np.float32))

    nc = _get_graph()
    in_maps = []
    for c in range(N_CORES):
        in_maps.append({
            "x": x[c * B:(c + 1) * B],
            "Wq": Wq, "Wk": Wk, "Wv": Wv, "Wo": Wo,
            "norm_const": norm_const,
        })
    res = run_bass_kernel_spmd(nc, in_maps, list(range(N_CORES)),
                               trace=_trace)
    out = np.concatenate([res.results[c]["out"] for c in range(N_CORES)],
                         axis=0)
    if _trace:
        kernel.last_exec_time_ns = res.exec_time_ns
        kernel.last_results = res
    return out


# revision 28
# speedup vs baseline: 1.1874x; 1.0676x over previous
"""Trainium2 Bass kernel for cosine linear-attention (nn_Attention).

Data-parallel over batch N=16 across 8 NeuronCores (2 batches/core,
weights replicated, no collectives). Per core:

  q = l2norm(x @ Wq.T), k = l2norm(x @ Wk.T), v = (x @ Wv.T) * C^-sigmoid(nc)
  out = (q @ (k^T v per head)) @ Wo.T

Compute runs in bf16 on the TensorEngine (1 cyc/row vs 4 for f32), f32
PSUM accumulation. x and the weights are cast f32->bf16 via SWDGE DMA
into DRAM scratch, then DMA-transposed (X-bar) into SBUF so the
contraction dim (d) lands on partitions. The k-l2norm scale and the
per-head v scale are folded into v; q's l2norm scale is applied to q
before its PE transpose into q^T layout for the attention matmuls.
"""

import sys

for _p in ("/opt/trn_rl_repo",):
    if _p not in sys.path:
        sys.path.append(_p)

import numpy as np
from contextlib import ExitStack

import concourse.bass as bass
import concourse.tile as tile
from concourse import bacc, mybir
from concourse.masks import make_identity
from concourse.bass_utils import run_bass_kernel_spmd

F32 = mybir.dt.float32
BF16 = mybir.dt.bfloat16

N_CORES = 8
N, C, D = 16, 1024, 1024
H, HD = 16, 64
B = N // N_CORES          # batches per core
P = 128
KC = D // P               # contraction chunks (8)
CT = C // P               # c tiles per batch (8)
MC = D // 512             # 512-wide m chunks (2)
HP = H // 2               # head pairs (8)
LN_C = float(np.log(C))


def build_graph():
    nc = bacc.Bacc("TRN2", target_bir_lowering=False, debug=False,
                   num_devices=N_CORES)
    x_ext = nc.declare_dram_parameter("x", [B, C, D], F32, isOutput=False)
    w_ext = {
        w: nc.declare_dram_parameter(w, [D, D], F32, isOutput=False)
        for w in ("Wq", "Wk", "Wv", "Wo")
    }
    ncst_ext = nc.declare_dram_parameter("norm_const", [1, H, 1, 1], F32,
                                         isOutput=False)
    out_ext = nc.declare_dram_parameter("out", [B, C, D], F32, isOutput=True)

    with tile.TileContext(nc) as tc, ExitStack() as ctx:
        singles = ctx.enter_context(tc.tile_pool(name="singles", bufs=1))
        dram = ctx.enter_context(tc.tile_pool(name="dram", bufs=1,
                                              space="DRAM"))
        ld_pool = ctx.enter_context(tc.tile_pool(name="ld", bufs=2))
        cast_pool = ctx.enter_context(tc.tile_pool(name="cast", bufs=2))
        wt_pool = ctx.enter_context(tc.tile_pool(name="wt", bufs=1))
        xt_pool = ctx.enter_context(tc.tile_pool(name="xt", bufs=1))
        kvq_pool = ctx.enter_context(tc.tile_pool(name="kvq", bufs=2))
        sq_pool = ctx.enter_context(tc.tile_pool(name="sq", bufs=2))
        stat_pool = ctx.enter_context(tc.tile_pool(name="stat", bufs=2))
        qt_pool = ctx.enter_context(tc.tile_pool(name="qt", bufs=1))
        at_pool = ctx.enter_context(tc.tile_pool(name="at", bufs=1))
        bd_pool = ctx.enter_context(tc.tile_pool(name="bd", bufs=2))
        out_pool = ctx.enter_context(tc.tile_pool(name="osb", bufs=3))
        proj_psum = ctx.enter_context(
            tc.tile_pool(name="proj_psum", bufs=6, space="PSUM"))
        kv_sb_pool = ctx.enter_context(tc.tile_pool(name="kvacc", bufs=2))
        tp_psum = ctx.enter_context(
            tc.tile_pool(name="tp_psum", bufs=2, space="PSUM"))

        # ---- prologue: per-head v scale C^-sigmoid(norm_const) -> [128, H]
        svec = singles.tile([1, H], F32, name="svec")
        nc.sync.dma_start(out=svec[:], in_=ncst_ext[0, :, 0, 0])
        ssig = singles.tile([1, H], F32, name="ssig")
        nc.scalar.activation(ssig[:], svec[:],
                             mybir.ActivationFunctionType.Sigmoid)
        sexp = singles.tile([1, H], F32, name="sexp")
        nc.scalar.activation(sexp[:], ssig[:],
                             mybir.ActivationFunctionType.Exp, scale=-LN_C)
        sv128 = singles.tile([P, H], F32, name="sv128")
        nc.gpsimd.partition_broadcast(sv128[:], sexp[0:1, :])

        ident = singles.tile([P, P], BF16, name="ident")
        make_identity(nc, ident[:])

        # ---- weights into SBUF transposed: wt[w][kc] = W.T[kc*128:.., :].
        # Wk goes through the PE (load f32 rows, ACT cast, PE transpose) so
        # the K projections can start within a few us; Wv/Wq/Wo take the
        # DMA route (whole-W SWDGE cast to DRAM bf16, then X-bar
        # DMA-transpose), overlapped under the K pass compute.
        wt = {
            w: wt_pool.tile([P, KC, D], BF16, name=f"wt_{w}", tag=f"wt_{w}")
            for w in ("Wk", "Wv", "Wq", "Wo")
        }
        for wname in ("Wk",):
            for mt in range(KC):
                wf = ld_pool.tile([P, D], F32, name="wf", tag="wf", bufs=3)
                nc.sync.dma_start(out=wf[:],
                                  in_=w_ext[wname][mt * P:(mt + 1) * P, :])
                wb = cast_pool.tile([P, D], BF16, name="wb", tag="wb", bufs=3)
                nc.scalar.copy(wb[:], wf[:])
                for g in range(2):
                    pst = tp_psum.tile([P, 512], BF16, name="wpst", tag="pst")
                    for j in range(4):
                        kc = g * 4 + j
                        nc.tensor.transpose(pst[:, j * P:(j + 1) * P],
                                            wb[:, kc * P:(kc + 1) * P],
                                            ident[:])
                    nc.vector.tensor_copy(
                        wt[wname][:, g * 4:(g + 1) * 4,
                                  mt * P:(mt + 1) * P],
                        pst[:].rearrange("p (j m) -> p j m", j=4))

        def dma_weight(wname):
            # whole-W SWDGE cast to DRAM bf16, then X-bar transposes; emitted
            # late so its HBM traffic stays off the startup critical path
            wbf = dram.tile([D, D], BF16, name=f"wbf_{wname}",
                            tag=f"wbf_{wname}")
            nc.gpsimd.dma_start(out=wbf[:], in_=w_ext[wname][:, :])
            for kc in range(KC):
                nc.sync.dma_start(out=wt[wname][:, kc, :],
                                  in_=wbf[:, kc * P:(kc + 1) * P],
                                  transpose=True)

        for n in range(B):
            # ---- x: same load/cast/PE-transpose; xt[kc] = x[n].T chunk
            xt = xt_pool.tile([P, KC, C], BF16, name="xt", tag="xt")

            def load_x_tile(ct_):
                xf = ld_pool.tile([P, D], F32, name="xf", tag="xf", bufs=4)
                nc.scalar.dma_start(out=xf[:],
                                    in_=x_ext[n, ct_ * P:(ct_ + 1) * P, :])
                xb = cast_pool.tile([P, D], BF16, name="xb", tag="xb", bufs=3)
                nc.scalar.copy(xb[:], xf[:])
                for g in range(2):
                    pst = tp_psum.tile([P, 512], BF16, name="xpst", tag="pst")
                    for j in range(4):
                        kc = g * 4 + j
                        nc.tensor.transpose(pst[:, j * P:(j + 1) * P],
                                            xb[:, kc * P:(kc + 1) * P],
                                            ident[:])
                    nc.vector.tensor_copy(
                        xt[:, g * 4:(g + 1) * 4, ct_ * P:(ct_ + 1) * P],
                        pst[:].rearrange("p (j m) -> p j m", j=4))

            # per-(head-pair) kv accumulators in SBUF f32 (PSUM accumulation
            # groups can't interleave within a bank: start=True clears
            # has_written for the whole 2KB zero region). Each c tile's kv
            # partial is a start+stop matmul into PSUM, then DVE-added here.
            kvsb = [
                kv_sb_pool.tile([P, 512], F32, name=f"kvsb_{b}",
                                tag=f"kvsb_{b}")
                for b in range(2)
            ]

            # q^T strips, written chunk-by-chunk across the c loop
            qt = qt_pool.tile([P, KC, C], BF16, name="qt", tag="qt")

            # ---- phase A helpers
            def project(wname, ct, pname):
                cs = slice(ct * P, (ct + 1) * P)
                ps = {}
                for mc in range(MC):
                    ps[mc] = proj_psum.tile([P, 512], F32,
                                            name=f"ps{pname}_{mc}",
                                            tag="proj")
                for kc in range(KC):
                    for mc in range(MC):
                        nc.tensor.matmul(
                            ps[mc][:],
                            xt[:, kc, cs],
                            wt[wname][:, kc, mc * 512:(mc + 1) * 512],
                            start=(kc == 0),
                            stop=(kc == KC - 1),
                        )
                return ps

            def group_sumsq(ps, ssname):
                ss = stat_pool.tile([P, H], F32, name=ssname, tag=ssname)
                for mc in range(MC):
                    sq = sq_pool.tile([P, 512], F32, name="sq", tag="sq")
                    nc.scalar.square(sq[:], ps[mc][:])
                    nc.vector.tensor_reduce(
                        ss[:, mc * 8:(mc + 1) * 8],
                        sq[:].rearrange("p (g d) -> p g d", g=8),
                        mybir.AxisListType.X,
                        mybir.AluOpType.add,
                    )
                return ss

            def rsqrt_(ss, rname):
                r = stat_pool.tile([P, H], F32, name=rname, tag=rname)
                nc.vector.tensor_scalar_max(r[:], ss[:], 1e-30)
                nc.vector.reciprocal(r[:], r[:])
                nc.scalar.sqrt(r[:], r[:])
                return r

            def scaled_to_bf16(ps, r, outname, tag=None):
                o = kvq_pool.tile([P, D], BF16, name=outname, tag=tag or outname)
                for mc in range(MC):
                    ms = slice(mc * 512, (mc + 1) * 512)
                    nc.vector.tensor_mul(
                        o[:, ms].rearrange("p (g d) -> p g d", g=8),
                        ps[mc][:].rearrange("p (g d) -> p g d", g=8),
                        r[:, mc * 8:(mc + 1) * 8][:, :, None]
                        .broadcast_to((P, 8, HD)),
                    )
                return o

            # ---- phase A-K: K projections (raw bf16; l2norm folded into v)
            # x tile ct's load/cast/transpose is emitted right before the
            # K matmuls that consume it, so the PE's static instruction
            # order lets the first projections start as soon as tile 0 and
            # Wk have landed.
            ksbs, ssks = [], []
            for ct in range(CT):
                load_x_tile(ct)
                psK = project("Wk", ct, "K")
                ssks.append(group_sumsq(psK, f"ssk_{ct}"))
                ksb = kvq_pool.tile([P, D], BF16, name=f"ksb_{ct}",
                                    tag=f"ksb_{ct}", bufs=1)
                for mc in range(MC):
                    ms = slice(mc * 512, (mc + 1) * 512)
                    nc.any.tensor_copy(ksb[:, ms], psK[mc][:])
                ksbs.append(ksb)

            if n == 0:
                with tc.tile_wait_until(0.022):
                    dma_weight("Wv")
                with tc.tile_wait_until(0.040):
                    dma_weight("Wq")

            # ---- phase A-V: V projections + kv partial accumulation
            for ct in range(CT):
                psV = project("Wv", ct, "V")
                rk = rsqrt_(ssks[ct], "rk")
                rkv = stat_pool.tile([P, H], F32, name="rkv", tag="rkv")
                nc.vector.tensor_mul(rkv[:], rk[:], sv128[:])
                vsb = scaled_to_bf16(psV, rkv, "vsb")
                for b in range(2):
                    kvp = proj_psum.tile([P, 512], F32, name=f"kvp_{b}",
                                         tag="proj")
                    for j in range(4):
                        hp = b * 4 + j
                        hs = slice(hp * P, (hp + 1) * P)
                        nc.tensor.matmul(
                            kvp[:, j * P:(j + 1) * P],
                            ksbs[ct][:, hs],
                            vsb[:, hs],
                            start=True,
                            stop=True,
                        )
                    if ct == 0:
                        nc.vector.tensor_copy(kvsb[b][:], kvp[:])
                    else:
                        nc.vector.tensor_add(kvsb[b][:], kvsb[b][:], kvp[:])

            if n == 0:
                with tc.tile_wait_until(0.060):
                    dma_weight("Wo")

            # ---- phase A-Q: Q projections + l2norm + PE transpose into q^T
            for ct in range(CT):
                cs = slice(ct * P, (ct + 1) * P)
                psQ = project("Wq", ct, "Q")
                ssq = group_sumsq(psQ, "ssq")
                rq = rsqrt_(ssq, "rq")
                qsb = scaled_to_bf16(psQ, rq, "qsb")
                for g in range(2):
                    pst = tp_psum.tile([P, 512], BF16, name="pst", tag="pst")
                    for j in range(4):
                        mt = g * 4 + j
                        nc.tensor.transpose(pst[:, j * P:(j + 1) * P],
                                            qsb[:, mt * P:(mt + 1) * P],
                                            ident[:])
                    nc.any.tensor_copy(
                        qt[:, g * 4:(g + 1) * 4, cs],
                        pst[:].rearrange("p (j m) -> p j m", j=4))

            # ---- phase B: block-diagonal kv tiles (off-diag junk zeroed)
            bds = []
            for hp in range(HP):
                kv = kvsb[hp // 4][:, (hp % 4) * P:(hp % 4 + 1) * P]
                bd = bd_pool.tile([P, P], BF16, name=f"bd_{hp}", tag="bd")
                nc.gpsimd.memset(bd[:], 0.0)
                nc.gpsimd.tensor_copy(bd[0:64, 0:64], kv[0:64, 0:64])
                nc.gpsimd.tensor_copy(bd[64:P, 64:P], kv[64:P, 64:P])
                bds.append(bd)

            # ---- phase C: attn^T = blockdiag(kv)^T-free @ q^T
            ats = []
            for hp in range(HP):
                at = at_pool.tile([P, C], BF16, name=f"at_{hp}",
                                  tag=f"at_{hp}")
                for cc in range(MC):
                    ccs = slice(cc * 512, (cc + 1) * 512)
                    psA = proj_psum.tile([P, 512], F32, name="psA", tag="proj")
                    nc.tensor.matmul(psA[:], bds[hp][:], qt[:, hp, ccs],
                                     start=True, stop=True)
                    nc.any.tensor_copy(at[:, ccs], psA[:])
                ats.append(at)

            # ---- phase D: out = attn^T.T @ Wo.T (one stationary load per
            # (ct, hp) feeds both 512-wide m chunks)
            for ct in range(CT):
                cs = slice(ct * P, (ct + 1) * P)
                psO = [
                    proj_psum.tile([P, 512], F32, name=f"psO_{mc}",
                                   tag="proj")
                    for mc in range(MC)
                ]
                for hp in range(HP):
                    for mc in range(MC):
                        nc.tensor.matmul(
                            psO[mc][:],
                            ats[hp][:, cs],
                            wt["Wo"][:, hp, mc * 512:(mc + 1) * 512],
                            start=(hp == 0),
                            stop=(hp == HP - 1),
                        )
                for mc in range(MC):
                    ms = slice(mc * 512, (mc + 1) * 512)
                    osb = out_pool.tile([P, 512], F32, name="osb", tag="osb")
                    nc.any.tensor_copy(osb[:], psO[mc][:])
                    nc.sync.dma_start(out=out_ext[n, cs, ms], in_=osb[:])

    nc.compile()
    return nc


_NC_CACHE = None


def _get_graph():
    global _NC_CACHE
    if _NC_CACHE is None:
        _NC_CACHE = build_graph()
    return _NC_CACHE


def kernel(x, Wq, Wk, Wv, Wo, norm_const, _trace=False):
    x = np.ascontiguousarray(np.asarray(x, dtype=np.float32))
    Wq = np.ascontiguousarray(np.asarray(Wq, dtype=np.float32))
    Wk = np.ascontiguousarray(np.asarray(Wk, dtype=np.float32))
    Wv = np.ascontiguousarray(np.asarray(Wv, dtype=np.float32))
    Wo = np.ascontiguousarray(np.asarray(Wo, dtype=np.float32))
    norm_const = np.ascontiguousarray(np.asarray(norm_const, dtype=np.float32))

    nc = _get_graph()
    in_maps = []
    for c in range(N_CORES):
        in_maps.append({
            "x": x[c * B:(c + 1) * B],
            "Wq": Wq, "Wk": Wk, "Wv": Wv, "Wo": Wo,
            "norm_const": norm_const,
        })
    res = run_bass_kernel_spmd(nc, in_maps, list(range(N_CORES)),
                               trace=_trace)
    out = np.concatenate([res.results[c]["out"] for c in range(N_CORES)],
                         axis=0)
    if _trace:
        kernel.last_exec_time_ns = res.exec_time_ns
        kernel.last_results = res
    return out
